# revision 6
# baseline (speedup 1.0000x reference)
"""Trainium2 Bass kernel for additive (Bahdanau-style) attention aggregation.

Reference computation per batch b:
    qe = query @ Wq + bq                       # [Lq, D]
    me = memory @ Wm + bm                      # [Lm, D]
    S[q,m] = sum_d wst[d] * tanh(qe[q,d] + me[m,d])
    S = softmax(mask ? S : -inf, axis=m)
    out = S @ memory                           # [Lq, D]

Sharding: data-parallel over batch B=8, one batch element per NeuronCore.

Per-core design (d = 512 split into 4 chunks of 128 partitions):
  - PE computes qe_T [d,q] and me_T [d,m] via matmuls on transposed inputs
    (inputs transposed on-chip via PE identity transpose). bq+bm folded
    into qe_T.
  - DVE builds item[d, q*256+m] = me_T[d,m] + qe_T[d,q] via per-(q,chunk)
    tensor_scalar_add (fp32 hits the 2x_2P perf mode).
  - ACT applies tanh over long [128, QB*256] tiles (the bottleneck engine:
    ~1 elem/cycle/lane, so long tiles amortize the fixed overhead).
  - PE reduces over d with wst via matmuls whose stationary is a [128,256]
    zero tile with wst chunk at column 128, sliced [:, 128-q:256-q] so the
    result lands on PSUM row q. All 512 matmuls accumulate into one PSUM
    tile S[q,m], which is already in softmax-friendly layout.
  - Softmax on DVE/ACT, then out = P @ memory on PE.
"""

import numpy as np

import concourse.bass as bass
import concourse.bacc as bacc
import concourse.tile as tile
from concourse import mybir
from concourse.bass_utils import run_bass_kernel_spmd
from concourse.masks import make_identity

F32 = mybir.dt.float32
U8 = mybir.dt.uint8
AF = mybir.ActivationFunctionType
AX = mybir.AxisListType
OP = mybir.AluOpType

B = 8          # batch, one per core
LQ = 128       # query length
LM = 256       # memory length
D = 512        # d_model == d_query == d_memory
KC = D // 128  # partition chunks of the d dimension
MH = LM // 128 # memory partition chunks
QB = 16        # q-block per tanh instruction
NQB = LQ // QB


def _build() -> bass.Bass:
    nc = bacc.Bacc("TRN2", target_bir_lowering=False)

    q_d = nc.declare_dram_parameter("query", [LQ, D], F32, isOutput=False)
    m_d = nc.declare_dram_parameter("memory", [LM, D], F32, isOutput=False)
    wq_d = nc.declare_dram_parameter("Wq", [D, D], F32, isOutput=False)
    bq_d = nc.declare_dram_parameter("bq", [D], F32, isOutput=False)
    wm_d = nc.declare_dram_parameter("Wm", [D, D], F32, isOutput=False)
    bm_d = nc.declare_dram_parameter("bm", [D], F32, isOutput=False)
    wst_d = nc.declare_dram_parameter("wst", [D], F32, isOutput=False)
    mask_d = nc.declare_dram_parameter("mask", [LM], U8, isOutput=False)
    out_d = nc.declare_dram_parameter("out", [LQ, D], F32, isOutput=True)

    with tile.TileContext(nc) as tc:
        with (
            tc.tile_pool(name="const", bufs=1) as const,
            tc.tile_pool(name="wts", bufs=1) as wts,
            tc.tile_pool(name="io", bufs=1) as io,
            tc.tile_pool(name="item_p", bufs=3) as item_p,
            tc.tile_pool(name="act_p", bufs=3) as act_p,
            tc.tile_pool(name="ps_misc", bufs=3, space="PSUM") as ps_misc,
            tc.tile_pool(name="ps_s", bufs=1, space="PSUM") as ps_s,
            tc.tile_pool(name="ps_out", bufs=1, space="PSUM") as ps_out,
        ):
            # ---- constants / small loads --------------------------------
            ident = const.tile([128, 128], F32, tag="ident")
            make_identity(nc, ident[:])

            # preload the tanh/exp activation table while DMAs run
            dummy = const.tile([128, 1], F32, tag="dummy")
            nc.vector.memset(dummy[:], 0.0)
            nc.scalar.activation(dummy[:], dummy[:], AF.Tanh)

            bqT = const.tile([128, KC], F32, tag="bqT")
            nc.gpsimd.dma_start(bqT[:], bq_d[:].rearrange("(c p) -> p c", p=128))
            bmT = const.tile([128, KC], F32, tag="bmT")
            nc.gpsimd.dma_start(bmT[:], bm_d[:].rearrange("(c p) -> p c", p=128))
            wstT = const.tile([128, KC], F32, tag="wstT")
            nc.gpsimd.dma_start(wstT[:], wst_d[:].rearrange("(c p) -> p c", p=128))
            bsum = const.tile([128, KC], F32, tag="bsum")
            nc.vector.tensor_add(bsum[:], bqT[:], bmT[:])

            # gpsimd DMA can cast u8 -> f32 during the broadcast load
            mask_f = const.tile([128, LM], F32, tag="mask_f")
            nc.gpsimd.dma_start(mask_f[:], mask_d[:].partition_broadcast(128))

            # W2[c]: zeros except column 128 = wst chunk c. The stationary
            # slice W2[c][:, 128-q:256-q] then has wst in column q only.
            W2 = []
            for c in range(KC):
                w2c = const.tile([128, 2 * 128], F32, tag=f"w2_{c}")
                nc.vector.memset(w2c[:], 0.0)
                nc.vector.tensor_copy(w2c[:, 128:129], wstT[:, c : c + 1])
                W2.append(w2c)

            # ---- input loads --------------------------------------------
            q_sb = io.tile([128, D], F32, tag="q_sb")
            nc.gpsimd.dma_start(q_sb[:], q_d[:])
            mem_sb = []
            for h in range(MH):
                t = io.tile([128, D], F32, tag=f"mem_{h}")
                nc.gpsimd.dma_start(t[:], m_d[h * 128 : (h + 1) * 128, :])
                mem_sb.append(t)

            # weights, DMA'd per [128,128] block, c-major so chunk 0's
            # columns land first and the encoder pipeline starts early
            wq_sb = [wts.tile([128, D], F32, tag=f"wq_{k}", name=f"wq_{k}") for k in range(KC)]
            wm_sb = [wts.tile([128, D], F32, tag=f"wm_{k}", name=f"wm_{k}") for k in range(KC)]
            for c in range(KC):
                cs = slice(c * 128, (c + 1) * 128)
                for k in range(KC):
                    ks = slice(k * 128, (k + 1) * 128)
                    nc.gpsimd.dma_start(wq_sb[k][:, cs], wq_d[ks, cs])
                    nc.gpsimd.dma_start(wm_sb[k][:, cs], wm_d[ks, cs])

            # ---- transpose query and memory (PE identity transpose) -----
            qT = []
            for c in range(KC):
                pst = ps_misc.tile([128, 256], F32, tag="ps_misc")
                nc.tensor.transpose(
                    pst[:, :128], q_sb[:, c * 128 : (c + 1) * 128], ident[:]
                )
                t = io.tile([128, 128], F32, tag=f"qT_{c}")
                nc.vector.tensor_copy(t[:], pst[:, :128])
                qT.append(t)
            mT_pre = [io.tile([128, LM], F32, tag=f"mT_{c}", name=f"mT_{c}") for c in range(KC)]
            for h in range(MH):
                hs = slice(h * 128, (h + 1) * 128)
                for c in range(KC):
                    pst = ps_misc.tile([128, 256], F32, tag="ps_misc")
                    nc.tensor.transpose(
                        pst[:, :128], mem_sb[h][:, c * 128 : (c + 1) * 128], ident[:]
                    )
                    nc.vector.tensor_copy(mT_pre[c][:, hs], pst[:, :128])

            # ---- encoders: qe_T = Wq^T @ query^T (+bq+bm), me_T = Wm^T @ mem^T
            qeTb = []
            for c in range(KC):
                cs = slice(c * 128, (c + 1) * 128)
                ps = ps_misc.tile([128, 256], F32, tag="ps_misc")
                for k in range(KC):
                    nc.tensor.matmul(
                        ps[:, :128],
                        wq_sb[k][:, cs],
                        qT[k][:],
                        start=(k == 0),
                        stop=(k == KC - 1),
                    )
                t = io.tile([128, 128], F32, tag=f"qeTb_{c}")
                nc.vector.tensor_scalar_add(t[:], ps[:, :128], bsum[:, c : c + 1])
                qeTb.append(t)
            meT = []
            for c in range(KC):
                cs = slice(c * 128, (c + 1) * 128)
                ps = ps_misc.tile([128, 256], F32, tag="ps_misc")
                for k in range(KC):
                    nc.tensor.matmul(
                        ps[:],
                        wm_sb[k][:, cs],
                        mT_pre[k][:],
                        start=(k == 0),
                        stop=(k == KC - 1),
                    )
                t = io.tile([128, LM], F32, tag=f"meT_{c}")
                nc.vector.tensor_copy(t[:], ps[:])
                meT.append(t)

            # ---- main loop: item -> tanh -> wst-reduce into S ----------
            s_ps = ps_s.tile([128, LM], F32, tag="s_ps")
            for qb in range(NQB):
                for c in range(KC):
                    it = item_p.tile([128, QB * LM], F32, tag="item")
                    for qi in range(QB):
                        q = qb * QB + qi
                        nc.vector.tensor_scalar_add(
                            it[:, qi * LM : (qi + 1) * LM],
                            meT[c][:],
                            qeTb[c][:, q : q + 1],
                        )
                    at = act_p.tile([128, QB * LM], F32, tag="act")
                    nc.scalar.activation(at[:], it[:], AF.Tanh)
                    for qi in range(QB):
                        q = qb * QB + qi
                        nc.tensor.matmul(
                            s_ps[:],
                            W2[c][:, 128 - q : 256 - q],
                            at[:, qi * LM : (qi + 1) * LM],
                            start=(qb == 0 and c == 0 and qi == 0),
                            stop=(qb == NQB - 1 and c == KC - 1 and qi == QB - 1),
                        )

            # ---- masked softmax over m ---------------------------------
            negmax = io.tile([128, 1], F32, tag="negmax")
            nc.vector.tensor_reduce(
                negmax[:], s_ps[:], axis=AX.X, op=OP.max, negate=True
            )
            expm = io.tile([128, LM], F32, tag="expm")
            nc.scalar.activation(expm[:], s_ps[:], AF.Exp, bias=negmax[:])
            expmm = io.tile([128, LM], F32, tag="expmm")
            nc.vector.tensor_mul(expmm[:], expm[:], mask_f[:])
            rsum = io.tile([128, 1], F32, tag="rsum")
            nc.vector.tensor_reduce(rsum[:], expmm[:], axis=AX.X, op=OP.add)
            rinv = io.tile([128, 1], F32, tag="rinv")
            nc.vector.reciprocal(rinv[:], rsum[:])
            p_sb = io.tile([128, LM], F32, tag="p_sb")
            nc.vector.tensor_scalar_mul(p_sb[:], expmm[:], rinv[:])

            # ---- out = P @ memory --------------------------------------
            pT = []
            for h in range(MH):
                pst = ps_misc.tile([128, 256], F32, tag="ps_misc")
                nc.tensor.transpose(
                    pst[:, :128], p_sb[:, h * 128 : (h + 1) * 128], ident[:]
                )
                t = io.tile([128, 128], F32, tag=f"pT_{h}")
                nc.vector.tensor_copy(t[:], pst[:, :128])
                pT.append(t)
            o_ps = ps_out.tile([128, D], F32, tag="o_ps")
            for h in range(MH):
                nc.tensor.matmul(
                    o_ps[:], pT[h][:], mem_sb[h][:], start=(h == 0), stop=(h == MH - 1)
                )
            o_sb = io.tile([128, D], F32, tag="o_sb")
            nc.vector.tensor_copy(o_sb[:], o_ps[:])
            nc.gpsimd.dma_start(out_d[:], o_sb[:])

    nc.compile()
    return nc


_NC = None


def _get_nc() -> bass.Bass:
    global _NC
    if _NC is None:
        _NC = _build()
    return _NC


def _make_in_maps(inputs):
    query = np.ascontiguousarray(np.asarray(inputs["query"], dtype=np.float32))
    memory = np.ascontiguousarray(np.asarray(inputs["memory"], dtype=np.float32))
    Wq = np.ascontiguousarray(np.asarray(inputs["Wq"], dtype=np.float32))
    bq = np.ascontiguousarray(np.asarray(inputs["bq"], dtype=np.float32))
    Wm = np.ascontiguousarray(np.asarray(inputs["Wm"], dtype=np.float32))
    bm = np.ascontiguousarray(np.asarray(inputs["bm"], dtype=np.float32))
    wst = np.ascontiguousarray(np.asarray(inputs["wst"], dtype=np.float32))
    mask = np.ascontiguousarray(np.asarray(inputs["memory_mask"]).astype(np.uint8))
    return [
        {
            "query": query[b],
            "memory": memory[b],
            "Wq": Wq,
            "bq": bq,
            "Wm": Wm,
            "bm": bm,
            "wst": wst,
            "mask": mask[b],
        }
        for b in range(B)
    ]


def run_raw(inputs, **kwargs):
    """Run and return the full BassKernelResults (for profiling from test.py)."""
    nc = _get_nc()
    return run_bass_kernel_spmd(nc, _make_in_maps(inputs), list(range(B)), **kwargs)


def kernel(**inputs) -> np.ndarray:
    res = run_raw(inputs)
    return np.stack([res.results[b]["out"] for b in range(B)]).astype(np.float32)


if __name__ == "__main__":
    nc = _get_nc()
    print("built ok")


# revision 8
# speedup vs baseline: 1.6441x; 1.6441x over previous
"""Trainium2 Bass kernel for additive (Bahdanau-style) attention aggregation.

Reference computation per batch b:
    qe = query @ Wq + bq                       # [Lq, D]
    me = memory @ Wm + bm                      # [Lm, D]
    S[q,m] = sum_d wst[d] * tanh(qe[q,d] + me[m,d])
    S = softmax(mask ? S : -inf, axis=m)
    out = S @ memory                           # [Lq, D]

Sharding: data-parallel over batch B=8, one batch element per NeuronCore.

Per-core design (d = 512 split into 4 chunks of 128 partitions):
  - PE computes qe_T [d,q] and me_T [d,m] via bf16 matmuls on transposed
    inputs (inputs transposed on-chip via PE identity transpose). bq+bm
    folded into qe_T.
  - DVE builds item[d, q*256+m] = me_T[d,m] + qe_T[d,q] via per-(q,chunk)
    bf16 tensor_scalar_add (2x_1P perf mode).
  - ACT applies tanh over long [128, QB*256] tiles (the bottleneck engine:
    1 elem/cycle/lane dtype-independent, so long tiles amortize overhead).
  - PE reduces over d with wst via bf16 matmuls whose stationary is a
    [128,256] zero tile with the wst chunk at column 128 (or 127 for odd q,
    keeping the 2-byte slice offset 4B-aligned for fast weight load),
    sliced so the product lands on PSUM row q. All 512 matmuls accumulate
    into one PSUM tile S[q,m], already in softmax-friendly layout.
  - Softmax in fp32 on DVE/ACT, then out = P @ memory in fp32 on PE.
"""

import numpy as np

import concourse.bass as bass
import concourse.bacc as bacc
import concourse.tile as tile
from concourse import mybir
from concourse.bass_utils import run_bass_kernel_spmd
from concourse.masks import make_identity

F32 = mybir.dt.float32
BF16 = mybir.dt.bfloat16
U8 = mybir.dt.uint8
AF = mybir.ActivationFunctionType
AX = mybir.AxisListType
OP = mybir.AluOpType

B = 8          # batch, one per core
LQ = 128       # query length
LM = 256       # memory length
D = 512        # d_model == d_query == d_memory
KC = D // 128  # partition chunks of the d dimension
MH = LM // 128 # memory partition chunks
QB = 32        # q-block per tanh instruction
NQB = LQ // QB


def _build() -> bass.Bass:
    nc = bacc.Bacc("TRN2", target_bir_lowering=False)

    q_d = nc.declare_dram_parameter("query", [LQ, D], F32, isOutput=False)
    m_d = nc.declare_dram_parameter("memory", [LM, D], F32, isOutput=False)
    wq_d = nc.declare_dram_parameter("Wq", [D, D], F32, isOutput=False)
    bq_d = nc.declare_dram_parameter("bq", [D], F32, isOutput=False)
    wm_d = nc.declare_dram_parameter("Wm", [D, D], F32, isOutput=False)
    bm_d = nc.declare_dram_parameter("bm", [D], F32, isOutput=False)
    wst_d = nc.declare_dram_parameter("wst", [D], F32, isOutput=False)
    mask_d = nc.declare_dram_parameter("mask", [LM], U8, isOutput=False)
    out_d = nc.declare_dram_parameter("out", [LQ, D], F32, isOutput=True)

    with tile.TileContext(nc) as tc:
        with (
            tc.tile_pool(name="const", bufs=1) as const,
            tc.tile_pool(name="wts", bufs=1) as wts,
            tc.tile_pool(name="io", bufs=1) as io,
            tc.tile_pool(name="item_p", bufs=3) as item_p,
            tc.tile_pool(name="act_p", bufs=3) as act_p,
            tc.tile_pool(name="ps_misc", bufs=3, space="PSUM") as ps_misc,
            tc.tile_pool(name="ps_s", bufs=1, space="PSUM") as ps_s,
            tc.tile_pool(name="ps_out", bufs=1, space="PSUM") as ps_out,
        ):
            # ---- constants / small loads --------------------------------
            ident = const.tile([128, 128], F32, tag="ident")
            make_identity(nc, ident[:])

            # preload the tanh/exp activation table while DMAs run
            dummy = const.tile([128, 1], F32, tag="dummy")
            nc.vector.memset(dummy[:], 0.0)
            nc.scalar.activation(dummy[:], dummy[:], AF.Tanh)

            bqT = const.tile([128, KC], F32, tag="bqT")
            nc.gpsimd.dma_start(bqT[:], bq_d[:].rearrange("(c p) -> p c", p=128))
            bmT = const.tile([128, KC], F32, tag="bmT")
            nc.gpsimd.dma_start(bmT[:], bm_d[:].rearrange("(c p) -> p c", p=128))
            wstT = const.tile([128, KC], F32, tag="wstT")
            nc.gpsimd.dma_start(wstT[:], wst_d[:].rearrange("(c p) -> p c", p=128))
            bsum = const.tile([128, KC], F32, tag="bsum")
            nc.vector.tensor_add(bsum[:], bqT[:], bmT[:])

            # gpsimd DMA can cast u8 -> f32 during the broadcast load
            mask_f = const.tile([128, LM], F32, tag="mask_f")
            nc.gpsimd.dma_start(mask_f[:], mask_d[:].partition_broadcast(128))

            # W2[c]: zeros except column 128 = wst chunk c (bf16). The
            # stationary slice W2[c][:, 128-q:256-q] then has wst in column
            # q only. W2b has wst at column 127 and serves odd q via
            # [:, 127-q:255-q] so the byte offset stays 4B-aligned (FWL).
            W2, W2b = [], []
            for c in range(KC):
                w2c = const.tile([128, 2 * 128], BF16, tag=f"w2_{c}", name=f"w2_{c}")
                nc.vector.memset(w2c[:], 0.0)
                nc.vector.tensor_copy(w2c[:, 128:129], wstT[:, c : c + 1])
                W2.append(w2c)
                w2bc = const.tile([128, 2 * 128], BF16, tag=f"w2b_{c}", name=f"w2b_{c}")
                nc.vector.memset(w2bc[:], 0.0)
                nc.vector.tensor_copy(w2bc[:, 127:128], wstT[:, c : c + 1])
                W2b.append(w2bc)

            # ---- input loads --------------------------------------------
            q_sb = io.tile([128, D], F32, tag="q_sb")
            nc.gpsimd.dma_start(q_sb[:], q_d[:])
            mem_sb = []
            for h in range(MH):
                t = io.tile([128, D], F32, tag=f"mem_{h}", name=f"mem_{h}")
                nc.gpsimd.dma_start(t[:], m_d[h * 128 : (h + 1) * 128, :])
                mem_sb.append(t)

            # weights in bf16 (gpsimd DMA casts), per [128,128] block,
            # c-major so chunk 0's columns land first
            wq_sb = [wts.tile([128, D], BF16, tag=f"wq_{k}", name=f"wq_{k}") for k in range(KC)]
            wm_sb = [wts.tile([128, D], BF16, tag=f"wm_{k}", name=f"wm_{k}") for k in range(KC)]
            for c in range(KC):
                cs = slice(c * 128, (c + 1) * 128)
                for k in range(KC):
                    ks = slice(k * 128, (k + 1) * 128)
                    nc.gpsimd.dma_start(wq_sb[k][:, cs], wq_d[ks, cs])
                    nc.gpsimd.dma_start(wm_sb[k][:, cs], wm_d[ks, cs])

            # ---- transpose query and memory (PE identity transpose) -----
            qT = []
            for c in range(KC):
                pst = ps_misc.tile([128, 256], F32, tag="ps_misc")
                nc.tensor.transpose(
                    pst[:, :128], q_sb[:, c * 128 : (c + 1) * 128], ident[:]
                )
                t = io.tile([128, 128], BF16, tag=f"qT_{c}", name=f"qT_{c}")
                nc.vector.tensor_copy(t[:], pst[:, :128])
                qT.append(t)
            mT_pre = [
                io.tile([128, LM], BF16, tag=f"mT_{c}", name=f"mT_{c}")
                for c in range(KC)
            ]
            for h in range(MH):
                hs = slice(h * 128, (h + 1) * 128)
                for c in range(KC):
                    pst = ps_misc.tile([128, 256], F32, tag="ps_misc")
                    nc.tensor.transpose(
                        pst[:, :128], mem_sb[h][:, c * 128 : (c + 1) * 128], ident[:]
                    )
                    nc.vector.tensor_copy(mT_pre[c][:, hs], pst[:, :128])

            # ---- encoders: qe_T = Wq^T @ query^T (+bq+bm), me_T = Wm^T @ mem^T
            qeTb = []
            for c in range(KC):
                cs = slice(c * 128, (c + 1) * 128)
                ps = ps_misc.tile([128, 256], F32, tag="ps_misc")
                for k in range(KC):
                    nc.tensor.matmul(
                        ps[:, :128],
                        wq_sb[k][:, cs],
                        qT[k][:],
                        start=(k == 0),
                        stop=(k == KC - 1),
                    )
                t = io.tile([128, 128], F32, tag=f"qeTb_{c}", name=f"qeTb_{c}")
                nc.vector.tensor_scalar_add(t[:], ps[:, :128], bsum[:, c : c + 1])
                qeTb.append(t)
            meT = []
            for c in range(KC):
                cs = slice(c * 128, (c + 1) * 128)
                ps = ps_misc.tile([128, 256], F32, tag="ps_misc")
                for k in range(KC):
                    nc.tensor.matmul(
                        ps[:],
                        wm_sb[k][:, cs],
                        mT_pre[k][:],
                        start=(k == 0),
                        stop=(k == KC - 1),
                    )
                t = io.tile([128, LM], BF16, tag=f"meT_{c}", name=f"meT_{c}")
                nc.vector.tensor_copy(t[:], ps[:])
                meT.append(t)

            # ---- main loop: item -> tanh -> wst-reduce into S ----------
            s_ps = ps_s.tile([128, LM], F32, tag="s_ps")
            for qb in range(NQB):
                for c in range(KC):
                    it = item_p.tile([128, QB * LM], BF16, tag="item")
                    for qi in range(QB):
                        q = qb * QB + qi
                        nc.vector.tensor_scalar_add(
                            it[:, qi * LM : (qi + 1) * LM],
                            meT[c][:],
                            qeTb[c][:, q : q + 1],
                        )
                    at = act_p.tile([128, QB * LM], BF16, tag="act")
                    nc.scalar.activation(at[:], it[:], AF.Tanh)
                    for qi in range(QB):
                        q = qb * QB + qi
                        if q % 2 == 0:
                            lhsT = W2[c][:, 128 - q : 256 - q]
                        else:
                            lhsT = W2b[c][:, 127 - q : 255 - q]
                        nc.tensor.matmul(
                            s_ps[:],
                            lhsT,
                            at[:, qi * LM : (qi + 1) * LM],
                            start=(qb == 0 and c == 0 and qi == 0),
                            stop=(qb == NQB - 1 and c == KC - 1 and qi == QB - 1),
                        )

            # ---- masked softmax over m (fp32) ---------------------------
            negmax = io.tile([128, 1], F32, tag="negmax")
            nc.vector.tensor_reduce(
                negmax[:], s_ps[:], axis=AX.X, op=OP.max, negate=True
            )
            expm = io.tile([128, LM], F32, tag="expm")
            nc.scalar.activation(expm[:], s_ps[:], AF.Exp, bias=negmax[:])
            expmm = io.tile([128, LM], F32, tag="expmm")
            nc.vector.tensor_mul(expmm[:], expm[:], mask_f[:])
            rsum = io.tile([128, 1], F32, tag="rsum")
            nc.vector.tensor_reduce(rsum[:], expmm[:], axis=AX.X, op=OP.add)
            rinv = io.tile([128, 1], F32, tag="rinv")
            nc.vector.reciprocal(rinv[:], rsum[:])
            p_sb = io.tile([128, LM], F32, tag="p_sb")
            nc.vector.tensor_scalar_mul(p_sb[:], expmm[:], rinv[:])

            # ---- out = P @ memory (fp32) --------------------------------
            pT = []
            for h in range(MH):
                pst = ps_misc.tile([128, 256], F32, tag="ps_misc")
                nc.tensor.transpose(
                    pst[:, :128], p_sb[:, h * 128 : (h + 1) * 128], ident[:]
                )
                t = io.tile([128, 128], F32, tag=f"pT_{h}", name=f"pT_{h}")
                nc.vector.tensor_copy(t[:], pst[:, :128])
                pT.append(t)
            o_ps = ps_out.tile([128, D], F32, tag="o_ps")
            for h in range(MH):
                nc.tensor.matmul(
                    o_ps[:], pT[h][:], mem_sb[h][:], start=(h == 0), stop=(h == MH - 1)
                )
            o_sb = io.tile([128, D], F32, tag="o_sb")
            nc.vector.tensor_copy(o_sb[:], o_ps[:])
            nc.gpsimd.dma_start(out_d[:], o_sb[:])

    nc.compile()
    return nc


_NC = None


def _get_nc() -> bass.Bass:
    global _NC
    if _NC is None:
        _NC = _build()
    return _NC


def _make_in_maps(inputs):
    query = np.ascontiguousarray(np.asarray(inputs["query"], dtype=np.float32))
    memory = np.ascontiguousarray(np.asarray(inputs["memory"], dtype=np.float32))
    Wq = np.ascontiguousarray(np.asarray(inputs["Wq"], dtype=np.float32))
    bq = np.ascontiguousarray(np.asarray(inputs["bq"], dtype=np.float32))
    Wm = np.ascontiguousarray(np.asarray(inputs["Wm"], dtype=np.float32))
    bm = np.ascontiguousarray(np.asarray(inputs["bm"], dtype=np.float32))
    wst = np.ascontiguousarray(np.asarray(inputs["wst"], dtype=np.float32))
    mask = np.ascontiguousarray(np.asarray(inputs["memory_mask"]).astype(np.uint8))
    return [
        {
            "query": query[b],
            "memory": memory[b],
            "Wq": Wq,
            "bq": bq,
            "Wm": Wm,
            "bm": bm,
            "wst": wst,
            "mask": mask[b],
        }
        for b in range(B)
    ]


def run_raw(inputs, **kwargs):
    """Run and return the full BassKernelResults (for profiling from test.py)."""
    nc = _get_nc()
    return run_bass_kernel_spmd(nc, _make_in_maps(inputs), list(range(B)), **kwargs)


def kernel(**inputs) -> np.ndarray:
    res = run_raw(inputs)
    return np.stack([res.results[b]["out"] for b in range(B)]).astype(np.float32)


if __name__ == "__main__":
    nc = _get_nc()
    print("built ok")
